# revision 15
# baseline (speedup 1.0000x reference)
"""Trainium2 Bass kernel for NeuralCausalModel (per-variable 3-layer MLP).

Math (reference):
    wx = x @ A.T                                   [B, V]
    comb_i = concat([x, wx[:, i]], -1)             [B, V+1]
    h1_i = relu(comb_i @ W1[i].T + b1[i])          [B, D]
    h2_i = relu(h1_i @ W2[i].T + b2[i])            [B, D]
    out[:, i] = h2_i @ W3[i] + b3[i]               [B]

Host-side fold: the concat column contributes wx[b,i]*W1[i][d,V] with
wx[b,i] = sum_k x[b,k] A[i,k], so
    W1eff[i][d,k] = W1[i][d,k] + W1[i][d,V] * A[i,k]
    -> h1_i = relu(x @ W1eff[i].T + b1[i])
which removes the ragged K=257 contraction and the adjacency matmul.

Sharding: variable axis V=256 split across 8 cores (32 vars/core),
x replicated, out gathered on host. No collectives.

Device layout: activations transposed [feature, batch] so biases are
per-partition scalars. Per variable:
    M1: h1T[d,b] = relu(W1effT[i].T-chain @ xT)        16 matmuls N=512
    M2: ps2[e,b] = W2T[i]-chain @ h1T                  32 matmuls N=512
        q[e,b]   = w3[e] * max(ps2, -b2[e])            DVE fused epilogue
                 = w3[e]*relu(ps2+b2[e]) - w3[e]*b2[e]
    presum: acc[p,b] = sum_ee q[ee*128+p, b]           2 wide DVE adds
    M3: out[1,b] = ones.T @ acc + K_v                  2 matmuls N=512 (M=1)
        K_v = b3[v] + sum_e w3[e]*b2[e]                folds the max-trick
                                                       constant + bias
The fused epilogue keeps the W3 contraction off the TensorEngine except
for a single ones-reduce per batch half: PE work drops from 28672 to
25600 cycles/var. Stages are software-pipelined (M1(v) | M2(v-1) |
M3(v-2)) so ACT/DVE latency hides under PE time.
"""

import numpy as np

V, D, B = 256, 512, 1024
NCORES = 8
VL = V // NCORES  # 32 variables per core

_CACHE = {}


def _build():
    if "nc" in _CACHE:
        return _CACHE["nc"]

    import sys

    if "/opt/trn_rl_repo" not in sys.path:
        sys.path.insert(0, "/opt/trn_rl_repo")

    import concourse.mybir as mybir
    import concourse.tile as tile
    from concourse import bacc

    f32 = mybir.dt.float32
    f16 = mybir.dt.float16
    mdt = mybir.dt.float32r

    nc = bacc.Bacc("TRN2", target_bir_lowering=False, debug=False)

    xT = nc.declare_dram_parameter("xT", [V, B], mdt, isOutput=False)
    w1 = nc.declare_dram_parameter("w1t", [VL, V, D], mdt, isOutput=False)
    w2 = nc.declare_dram_parameter("w2t", [VL, D, D], mdt, isOutput=False)
    b1 = nc.declare_dram_parameter("b1t", [128, 128], f32, isOutput=False)
    nb2 = nc.declare_dram_parameter("nb2t", [128, 128], f32, isOutput=False)
    w3 = nc.declare_dram_parameter("w3t", [128, 128], f32, isOutput=False)
    kv = nc.declare_dram_parameter("kvt", [1, VL], f32, isOutput=False)
    ones = nc.declare_dram_parameter("onest", [128, 1], f16, isOutput=False)
    out = nc.declare_dram_parameter("out", [VL, B], f32, isOutput=True)

    Relu = mybir.ActivationFunctionType.Relu
    Ident = mybir.ActivationFunctionType.Identity
    add = mybir.AluOpType.add
    amax = mybir.AluOpType.max
    mult = mybir.AluOpType.mult

    with tile.TileContext(nc) as tc:
        with (
            tc.tile_pool(name="const", bufs=1) as const_pool,
            tc.tile_pool(name="w1p", bufs=6) as w1_pool,
            tc.tile_pool(name="w2p", bufs=12) as w2_pool,
            tc.tile_pool(name="h1p", bufs=12) as h1_pool,
            tc.tile_pool(name="qp", bufs=3) as q_pool,
            tc.tile_pool(name="orow", bufs=4) as orow_pool,
            tc.tile_pool(name="psp", bufs=4, space="PSUM") as ps_pool,
            tc.tile_pool(name="ps2p", bufs=2, space="PSUM") as ps2_pool,
        ):
            # Critical path first: x halves for bb=0 chains, then bb=1,
            # then b1 (first ACT epi). Secondary consts go via the DVE
            # queue so their DGE setup doesn't delay the weight DMAs.
            xt0 = const_pool.tile([128, B], mdt, tag="xt0")
            xt1 = const_pool.tile([128, B], mdt, tag="xt1")
            nc.sync.dma_start(xt0[:, 0:512], xT[0:128, 0:512])
            nc.sync.dma_start(xt1[:, 0:512], xT[128:256, 0:512])
            nc.sync.dma_start(xt0[:, 512:B], xT[0:128, 512:B])
            nc.sync.dma_start(xt1[:, 512:B], xT[128:256, 512:B])
            b1sb = const_pool.tile([128, 128], f32, tag="b1sb")
            nc.sync.dma_start(b1sb[:], b1[:])
            nb2sb = const_pool.tile([128, 128], f32, tag="nb2sb")
            nc.gpsimd.dma_start(nb2sb[:], nb2[:])
            w3sb = const_pool.tile([128, 128], f32, tag="w3sb")
            nc.gpsimd.dma_start(w3sb[:], w3[:])
            kvsb = const_pool.tile([1, VL], f32, tag="kvsb")
            nc.gpsimd.dma_start(kvsb[:], kv[:])
            ones16 = const_pool.tile([128, 1], f16, tag="ones16")
            nc.gpsimd.dma_start(ones16[:], ones[:])

            # Per-var state carried across pipeline stages.
            state = {}

            def emit_m1(v):
                w1t = [
                    w1_pool.tile([128, D], mdt, tag="w1t", name=f"w1t_{k}")
                    for k in range(2)
                ]
                for kk in range(2):
                    nc.sync.dma_start(w1t[kk][:], w1[v, kk * 128 : (kk + 1) * 128, :])
                w2t = [
                    w2_pool.tile([128, D], mdt, tag="w2t", name=f"w2t_{k}")
                    for k in range(4)
                ]
                for dd in range(4):
                    nc.sync.dma_start(w2t[dd][:], w2[v, dd * 128 : (dd + 1) * 128, :])

                h1t = [
                    h1_pool.tile([128, B], mdt, tag="h1t", name=f"h1t_{k}")
                    for k in range(4)
                ]
                xts = [xt0, xt1]
                for bb in range(2):
                    bs = slice(bb * 512, (bb + 1) * 512)
                    for dd in range(4):
                        ms = slice(dd * 128, (dd + 1) * 128)
                        ps = ps_pool.tile([128, 512], f32, tag="ps1", name="ps1")
                        nc.tensor.matmul(
                            ps[:], w1t[0][:, ms], xt0[:, bs], start=True, stop=False
                        )
                        nc.tensor.matmul(
                            ps[:], w1t[1][:, ms], xt1[:, bs], start=False, stop=True
                        )
                        nc.scalar.activation(
                            h1t[dd][:, bs],
                            ps[:],
                            Relu,
                            bias=b1sb[:, v * 4 + dd : v * 4 + dd + 1],
                        )
                state[v] = {"w2t": w2t, "h1t": h1t}

            def emit_m2(v):
                st = state[v]
                w2t, h1t = st["w2t"], st["h1t"]
                # q: [128, 4*B] fp16; segment ee at cols [ee*B, (ee+1)*B)
                qbig = q_pool.tile([128, 4 * B], f16, tag="qbig", name="qbig")
                for ee in range(4):
                    ms = slice(ee * 128, (ee + 1) * 128)
                    # Two banks; each 512-wide chain stays within one bank.
                    ps2 = ps2_pool.tile([128, 2 * 512], f32, tag="ps2", name="ps2")
                    for bb in range(2):
                        bs = slice(bb * 512, (bb + 1) * 512)
                        for dd in range(4):
                            nc.tensor.matmul(
                                ps2[:, bs],
                                w2t[dd][:, ms],
                                h1t[dd][:, bs],
                                start=(dd == 0),
                                stop=(dd == 3),
                            )
                    # q = w3 * max(z, -b2) = w3*relu(z+b2) - w3*b2
                    nc.vector.tensor_scalar(
                        qbig[:, ee * B : (ee + 1) * B],
                        ps2[:],
                        nb2sb[:, v * 4 + ee : v * 4 + ee + 1],
                        w3sb[:, v * 4 + ee : v * 4 + ee + 1],
                        op0=amax,
                        op1=mult,
                    )
                # presum over ee: 2 wide adds
                pr = q_pool.tile([128, 2 * B], f16, tag="pr", name="pr")
                nc.vector.tensor_tensor(
                    pr[:], qbig[:, 0 : 2 * B], qbig[:, 2 * B : 4 * B], op=add
                )
                acc = q_pool.tile([128, B], f16, tag="acc", name="acc")
                nc.vector.tensor_tensor(
                    acc[:], pr[:, 0:B], pr[:, B : 2 * B], op=add
                )
                st["acc"] = acc

            def emit_m3(v):
                acc = state[v]["acc"]
                orow = orow_pool.tile([1, B], f32, tag="orow", name="orow")
                for bb in range(2):
                    bs = slice(bb * 512, (bb + 1) * 512)
                    ps3 = ps_pool.tile([1, 512], f32, tag="ps1", name="ps3")
                    nc.tensor.matmul(
                        ps3[:], ones16[:], acc[:, bs], start=True, stop=True
                    )
                    nc.scalar.activation(
                        orow[0:1, bs],
                        ps3[:],
                        Ident,
                        bias=kvsb[0:1, v : v + 1],
                    )
                nc.sync.dma_start(out[v : v + 1, :], orow[:])
                del state[v]

            for i in range(VL + 2):
                if i == 1:
                    # Warmup: run M1(1) before M2(0) so the PE isn't
                    # waiting on h1(0) epilogues + w2(0) DMA.
                    emit_m1(1)
                    emit_m2(0)
                    continue
                if 1 <= i <= VL:
                    emit_m2(i - 1)
                if 2 <= i:
                    emit_m3(i - 2)
                if i < VL:
                    emit_m1(i)

    nc.compile()
    _CACHE["nc"] = nc
    return nc


def _prep_inputs(x, adjacency, W1, b1, W2, b2, W3, b3):
    """Host-side preprocessing + per-core sharding."""
    mmnp = np.float32  # float32r has fp32 storage
    x = np.asarray(x, np.float32)
    A = np.asarray(adjacency, np.float32)
    W1 = np.asarray(W1, np.float32)
    W2 = np.asarray(W2, np.float32)
    W3 = np.asarray(W3, np.float32)
    b1 = np.asarray(b1, np.float32)
    b2 = np.asarray(b2, np.float32)
    b3 = np.asarray(b3, np.float32)

    # Fold the concat column into W1.
    W1eff = W1[:, :, :V] + W1[:, :, V : V + 1] * A[:, None, :]  # [V, D, V]
    W1effT = np.ascontiguousarray(W1eff.transpose(0, 2, 1)).astype(mmnp)  # [i, k, d]
    W2T = np.ascontiguousarray(W2.transpose(0, 2, 1)).astype(mmnp)  # [i, d, e]
    xT = np.ascontiguousarray(x.T).astype(mmnp)  # [V, B]
    ones = np.ones((128, 1), np.float16)

    def col_tile(m):  # [VL, 512] -> [128, VL*4], col v*4+j, row p = m[v, j*128+p]
        return np.ascontiguousarray(
            m.reshape(VL, 4, 128).transpose(2, 0, 1).reshape(128, VL * 4)
        )

    in_maps = []
    for c in range(NCORES):
        s = slice(c * VL, (c + 1) * VL)
        kvt = (b3[s] + np.einsum("ve,ve->v", W3[s], b2[s])).reshape(1, VL)
        in_maps.append(
            {
                "xT": xT,
                "w1t": np.ascontiguousarray(W1effT[s]),
                "w2t": np.ascontiguousarray(W2T[s]),
                "b1t": col_tile(b1[s]),
                "nb2t": col_tile(-b2[s]),
                "w3t": col_tile(W3[s]),
                "kvt": np.ascontiguousarray(kvt.astype(np.float32)),
                "onest": ones,
                "out": np.zeros((VL, B), np.float32),
            }
        )
    return in_maps


def kernel(x, adjacency, W1, b1, W2, b2, W3, b3, _trace=False):
    import sys

    if "/opt/trn_rl_repo" not in sys.path:
        sys.path.insert(0, "/opt/trn_rl_repo")
    from concourse.bass_utils import run_bass_kernel_spmd

    nc = _build()
    in_maps = _prep_inputs(x, adjacency, W1, b1, W2, b2, W3, b3)
    for m in in_maps:
        m.pop("out", None)
    res = run_bass_kernel_spmd(
        nc, in_maps, core_ids=list(range(NCORES)), trace=_trace
    )
    kernel.last_results = res
    outT = np.concatenate([res.results[c]["out"] for c in range(NCORES)], axis=0)
    return np.ascontiguousarray(outT.T.astype(np.float32))


kernel.last_results = None
